# revision 25
# baseline (speedup 1.0000x reference)
"""Trainium2 Bass kernel for B=2,T=2048,C=2048,H=16 causal attention with RoPE.

Sharding: 8 NeuronCores = 2 (batch) x 4 (head-groups of 4 heads).
Each core computes q/k/v projections + RoPE + causal attention for its 4 heads
and a partial output projection over its 512 channels of Wo; the host sums the
4 partials per batch element.

Attention is computed in S^T layout (keys on partitions, queries on the free
axis): matmul(lhsT=K^T-block, rhs=Q^T-chunk) yields S^T directly, which is the
exact operand layout the A@V matmul needs — no PE transposes of the
probability matrix. The softmax denominator is accumulated per key-lane on the
vector engine and reduced+broadcast across partitions with a single
ones[128,128] matmul per query group; normalization happens after A@V on the
16x smaller output. The causal diagonal block is masked by a triangular 0/1
multiply on the vector engine after exp.
"""

import sys

for _p in ("/opt/trn_rl_repo",):
    if _p not in sys.path:
        sys.path.append(_p)


import math
from contextlib import ExitStack

import numpy as np
import ml_dtypes

import concourse.bass as bass
import concourse.mybir as mybir
import concourse.tile as tile

P = 128          # partitions / head dim / block size
T = 2048         # sequence length
C = 2048         # channels
NT = T // P      # 16 t-blocks
NCT = C // P     # 16 c-tiles
NH = 4           # heads per core
CHUNK = 512      # psum-bank-sized free chunk
NCH = T // CHUNK
QG = 4           # q-blocks per attention group
F32 = mybir.dt.float32
BF16 = mybir.dt.bfloat16
AF = mybir.ActivationFunctionType
AX = mybir.AxisListType


def split_sync_waits(nc, max_waits=1):
    """Walrus CoreV3 codegen rejects instructions carrying multiple sync-wait
    commands. Hoist extra waits onto NoOps inserted just before, on the same
    engine queue (same-engine program order preserves semantics)."""
    n = 0
    for bb in nc.main_func.blocks:
        new_list = []
        for inst in bb.instructions:
            si = inst.sync_info
            if si is not None and si.on_wait and len(si.on_wait) > max_waits:
                waits = list(si.on_wait)
                extra, keep = waits[:-max_waits], waits[-max_waits:]
                for w in extra:
                    n += 1
                    new_list.append(
                        mybir.InstNoOp(
                            name=f"splitw-{n}-{inst.name}",
                            engine=inst.engine,
                            sync_info=mybir.SyncInfo(on_wait=[w], on_update=[]),
                        )
                    )
                si.on_wait = keep
            new_list.append(inst)
        bb.instructions = new_list
    return n


def build_nc(split=True):
    nc = bass.Bass()

    xT = nc.dram_tensor("xT", [C, T], BF16, kind="ExternalInput")
    # wq/wk come pre-arranged [P, NH, NCT, P] so a head's slice DMAs as one
    # contiguous 4KB-per-partition block instead of 2048 256B descriptors
    wq = nc.dram_tensor("wq", [P, NH, NCT, P], BF16, kind="ExternalInput")
    wk = nc.dram_tensor("wk", [P, NH, NCT, P], BF16, kind="ExternalInput")
    wv = nc.dram_tensor("wv", [C, NH * P], BF16, kind="ExternalInput")
    wo = nc.dram_tensor("wo", [NH * P, C], BF16, kind="ExternalInput")
    cosk = nc.dram_tensor("cosk", [P, T], BF16, kind="ExternalInput")
    sink = nc.dram_tensor("sink", [P, T], BF16, kind="ExternalInput")
    rotm = nc.dram_tensor("rotm", [P, P], BF16, kind="ExternalInput")
    ones = nc.dram_tensor("ones", [P, P], BF16, kind="ExternalInput")
    ident = nc.dram_tensor("ident", [P, P], BF16, kind="ExternalInput")
    trilT = nc.dram_tensor("trilT", [P, P], BF16, kind="ExternalInput")
    y = nc.dram_tensor("y", [T, C], BF16, kind="ExternalOutput")

    with tile.TileContext(nc) as tc, ExitStack() as ctx:
        base = ctx.enter_context(tc.tile_pool(name="base", bufs=1))

        xt = base.tile([P, NCT, T], BF16, tag="xt")
        xTr = xT.rearrange("(n p) t -> p n t", p=P)

        cosk_sb = base.tile([P, T], BF16, tag="cosk")
        sink_sb = base.tile([P, T], BF16, tag="sink")
        rotm_sb = base.tile([P, P], BF16, tag="rotm")
        ones_sb = base.tile([P, P], BF16, tag="ones")
        ident_sb = base.tile([P, P], BF16, tag="ident")
        trilT_sb = base.tile([P, P], BF16, tag="trilT")

        wo_sb = base.tile([P, NH, C], BF16, tag="wo")

        v_all = base.tile([P, NT, NH * P], BF16, tag="v_all")
        attn_out = base.tile([P, NH, T], BF16, tag="attn_out")

        # ---- V projection: c-outer over two 8-t-block passes so compute
        # pipelines behind the streaming x DMA instead of waiting for all
        # 16 c-tiles before the first accumulation chain can finish ----
        with tc.tile_pool(name="wvp", bufs=1) as wvp, tc.tile_pool(
            name="pv", bufs=8, space="PSUM"
        ) as pv:
            wv_sb = wvp.tile([P, NCT, NH * P], BF16, tag="wv")
            wvr = wv.rearrange("(n p) m -> p n m", p=P)
            for c in range(NCT):
                nc.sync.dma_start(out=wv_sb[:, c, :], in_=wvr[:, c, :])
                nc.sync.dma_start(out=xt[:, c, :], in_=xTr[:, c, :])
            nc.sync.dma_start(out=cosk_sb, in_=cosk[:, :])
            nc.sync.dma_start(out=sink_sb, in_=sink[:, :])
            nc.sync.dma_start(out=rotm_sb, in_=rotm[:, :])
            nc.sync.dma_start(out=ones_sb, in_=ones[:, :])
            nc.sync.dma_start(out=ident_sb, in_=ident[:, :])
            nc.sync.dma_start(out=trilT_sb, in_=trilT[:, :])
            # gpsimd has no PSUM port, so only scalar/vector may drain psum
            copy_engines = [nc.scalar.copy, nc.vector.tensor_copy]
            # pass A: c-outer over t-blocks 0-7 so compute pipelines behind
            # the streaming x DMA
            vts = [
                pv.tile([P, CHUNK], F32, tag="v", name=f"v_psA_{i}")
                for i in range(8)
            ]
            for c in range(NCT):
                for i in range(8):
                    nc.tensor.matmul(
                        vts[i],
                        lhsT=xt[:, c, i * P:(i + 1) * P],
                        rhs=wv_sb[:, c, :],
                        start=(c == 0),
                        stop=(c == NCT - 1),
                    )
            for i in range(8):
                copy_engines[i % 2](out=v_all[:, i, :], in_=vts[i])
            # pass B: x is resident by now; tb-outer so the psum->sbuf
            # copies stagger instead of queueing at the end
            for tb in range(8, 16):
                vt = pv.tile([P, CHUNK], F32, tag="v", name=f"v_psB_{tb}")
                for c in range(NCT):
                    nc.tensor.matmul(
                        vt,
                        lhsT=xt[:, c, tb * P:(tb + 1) * P],
                        rhs=wv_sb[:, c, :],
                        start=(c == 0),
                        stop=(c == NCT - 1),
                    )
                copy_engines[tb % 2](out=v_all[:, tb, :], in_=vt)

        with tc.tile_pool(name="work", bufs=2) as work, tc.tile_pool(
            name="epool", bufs=6
        ) as epool, tc.tile_pool(name="wqk", bufs=2) as wqk_pool, tc.tile_pool(
            name="pproj", bufs=1, space="PSUM"
        ) as p_proj, tc.tile_pool(
            name="pqk", bufs=2, space="PSUM"
        ) as p_qk, tc.tile_pool(
            name="pav", bufs=2, space="PSUM"
        ) as p_av, tc.tile_pool(
            name="psmall", bufs=1, space="PSUM"
        ) as p_small:
            def load_w(h):
                wq_sb = wqk_pool.tile([P, NCT, P], BF16, tag="wq")
                nc.sync.dma_start(out=wq_sb, in_=wq[:, h, :, :])
                wk_sb = wqk_pool.tile([P, NCT, P], BF16, tag="wk")
                nc.sync.dma_start(out=wk_sb, in_=wk[:, h, :, :])
                return wq_sb, wk_sb

            def make_stageA(ch, w_tiles, ro_tiles):
                """Projection c-matmul closures for one 512-wide t-chunk of
                q and k (one PE matmul each), plus tail closures doing the
                RoPE rotate-matmul and combine (deferred so the raw copy
                latency hides under unrelated PE work)."""
                wq_sb, wk_sb = w_tiles
                qro, kro = ro_tiles
                sl = slice(ch * CHUNK, (ch + 1) * CHUNK)
                ops, tails = [], []
                for which, w_sb, ro in (("q", wq_sb, qro), ("k", wk_sb, kro)):
                    box = {}
                    for c in range(NCT):
                        def mm(c=c, which=which, w_sb=w_sb, box=box):
                            if c == 0:
                                box["ps"] = p_proj.tile(
                                    [P, CHUNK], F32, tag=which + "ps",
                                    name=which + "ps",
                                )
                            nc.tensor.matmul(
                                box["ps"],
                                lhsT=w_sb[:, c, :],
                                rhs=xt[:, c, sl],
                                start=(c == 0),
                                stop=(c == NCT - 1),
                            )
                            if c == NCT - 1:
                                box["raw"] = work.tile(
                                    [P, CHUNK], BF16, tag=which + "raw",
                                    name=which + "raw",
                                )
                                nc.scalar.copy(out=box["raw"], in_=box["ps"])
                        ops.append(mm)

                    def tail(which=which, ro=ro, box=box, sl=sl):
                        rot_ps = p_small.tile(
                            [P, CHUNK], F32, tag="rot", name="rot_ps"
                        )
                        nc.tensor.matmul(
                            rot_ps, lhsT=rotm_sb, rhs=box["raw"],
                            start=True, stop=True,
                        )
                        rot_bf = work.tile(
                            [P, CHUNK], BF16, tag=which + "rotbf", name="rot_bf"
                        )
                        nc.vector.tensor_copy(out=rot_bf, in_=rot_ps)
                        t1 = work.tile([P, CHUNK], BF16, tag=which + "t1", name="t1")
                        nc.vector.tensor_mul(out=t1, in0=box["raw"], in1=cosk_sb[:, sl])
                        t2 = work.tile([P, CHUNK], BF16, tag=which + "t2", name="t2")
                        nc.vector.tensor_mul(out=t2, in0=rot_bf, in1=sink_sb[:, sl])
                        nc.vector.tensor_add(out=ro[:, sl], in0=t1, in1=t2)
                    tails.append(tail)
                return ops, tails

            def make_wo_ops(g):
                """Output-projection closures for the 4 t-blocks of group g.
                Each closure emits the 4 head-contraction matmuls for one
                512-wide channel chunk plus the psum->sbuf copy; the last
                chunk of a t-block also issues the output DMA."""
                ops = []
                for tb in range(QG * g, QG * (g + 1)):
                    ybox = {}
                    for ch in range(NCH):
                        def op(tb=tb, ch=ch, ybox=ybox):
                            if ch == 0:
                                ybox["y"] = work.tile(
                                    [P, C], BF16, tag="y", name="y_sb"
                                )
                            y_ps = p_proj.tile(
                                [P, CHUNK], F32,
                                tag=("qps" if ch % 2 == 0 else "kps"),
                                name="y_ps",
                            )
                            for h2 in range(NH):
                                nc.tensor.matmul(
                                    y_ps,
                                    lhsT=attn_out[:, h2, tb * P:(tb + 1) * P],
                                    rhs=wo_sb[:, h2, ch * CHUNK:(ch + 1) * CHUNK],
                                    start=(h2 == 0),
                                    stop=(h2 == NH - 1),
                                )
                            # ACT engine: the DVE queue is the h=3 drumbeat
                            nc.scalar.copy(
                                out=ybox["y"][:, ch * CHUNK:(ch + 1) * CHUNK],
                                in_=y_ps,
                            )
                            if ch == NCH - 1:
                                nc.sync.dma_start(
                                    out=y[tb * P:(tb + 1) * P, :], in_=ybox["y"]
                                )
                        ops.append(op)
                return ops

            def attention_group(h, g, ro_tiles, fillers):
                """Causal attention for q-blocks [4g, 4g+4) of head h in S^T
                layout, with `fillers` (independent single-PE-op closures)
                interleaved into the PE stream so the engine never waits on
                the exp that sits between the QK^T and A@V matmuls."""
                qro, kro = ro_tiles
                hs = slice(h * P, (h + 1) * P)
                nkb = QG * (g + 1)
                qbase = g * QG * P
                nf = len(fillers)
                fi = 0
                acc = work.tile([P, CHUNK], F32, tag="lacc", name="lacc")
                av_ps = p_av.tile([P, CHUNK], F32, tag="av", name="av_ps")
                for kb in range(nkb):
                    j = kb - QG * g
                    q0 = max(j, 0) * P
                    s_ps = p_qk.tile([P, CHUNK], F32, tag="s", name="s_ps")
                    nc.tensor.matmul(
                        s_ps[:, q0:],
                        lhsT=kro[:, kb * P:(kb + 1) * P],
                        rhs=qro[:, qbase + q0:qbase + CHUNK],
                        start=True,
                        stop=(j < 0),
                        skip_group_check=True,
                    )
                    if j >= 0:  # diagonal block: -inf the non-causal triangle
                        nc.tensor.matmul(
                            s_ps[:, q0:q0 + P],
                            lhsT=ident_sb,
                            rhs=trilT_sb,
                            start=False,
                            stop=True,
                            skip_group_check=True,
                        )
                    et = epool.tile([P, CHUNK], BF16, tag="et", name="et")
                    nc.scalar.activation(
                        out=et[:, q0:],
                        in_=s_ps[:, q0:],
                        func=AF.Exp,
                        scale=float(P) ** -0.5,
                    )
                    if kb == 0:
                        nc.vector.tensor_copy(out=acc, in_=et)
                    else:
                        nc.vector.tensor_add(
                            out=acc[:, q0:], in0=acc[:, q0:], in1=et[:, q0:]
                        )
                    hi = (kb + 1) * nf // nkb
                    while fi < hi:
                        fillers[fi]()
                        fi += 1
                    nc.tensor.matmul(
                        av_ps[:, q0:],
                        lhsT=v_all[:, kb, hs],
                        rhs=et[:, q0:],
                        start=(kb == 0),
                        stop=(kb == nkb - 1),
                        skip_group_check=True,
                    )
                while fi < nf:
                    fillers[fi]()
                    fi += 1
                # softmax denominator finalize: per-lane sums -> reduce over
                # partitions + broadcast in one ones[128,128] matmul, then
                # normalize the A@V output. Returned as closures so the next
                # group's filler stream absorbs the serial chain instead of
                # stalling the PE at group end.
                box = {}

                def fin_cast():
                    # gpsimd: SBUF->SBUF, skips the busy DVE/ACT queues so
                    # the ones-matmul isn't stuck behind them
                    box["accb"] = work.tile([P, CHUNK], BF16, tag="accb", name="accb")
                    nc.gpsimd.tensor_copy(out=box["accb"], in_=acc)

                def fin_mm():
                    box["lb"] = p_small.tile([P, CHUNK], F32, tag="lbc", name="lb_ps")
                    nc.tensor.matmul(
                        box["lb"], lhsT=ones_sb, rhs=box["accb"], start=True, stop=True
                    )

                def fin_norm(blk):
                    # per-128-column piece: keeps the slow DVE reciprocal
                    # from monopolizing the FIFO and lets Wo consume each
                    # t-block of attn_out as soon as its piece lands
                    def op(blk=blk):
                        linv = work.tile(
                            [P, P], F32, tag="linv", bufs=4, name="linv"
                        )
                        bs = slice(blk * P, (blk + 1) * P)
                        nc.vector.reciprocal(out=linv, in_=box["lb"][:, bs])
                        nc.vector.tensor_mul(
                            out=attn_out[:, h, qbase + blk * P:qbase + (blk + 1) * P],
                            in0=av_ps[:, bs],
                            in1=linv,
                        )
                    return op

                return [fin_cast, fin_mm] + [fin_norm(b) for b in range(QG)]

            # ---- prologue: head-0 q/k projection + RoPE ----
            w_list = [None] * NH
            w_list[0] = load_w(0)
            qro0 = work.tile([P, T], BF16, tag="qro")
            kro0 = work.tile([P, T], BF16, tag="kro")
            ro_cur = (qro0, kro0)
            prev_tails = []
            for ch in range(NCH):
                a_ops, tails = make_stageA(ch, w_list[0], ro_cur)
                for op in a_ops[:2]:
                    op()
                for op in prev_tails:
                    op()
                for op in a_ops[2:]:
                    op()
                prev_tails = tails
            w_list[1] = load_w(1)

            def weave(fins, tails):
                """Order deferred ops of the previous group: the accb cast
                first (gpsimd, needs max lead), then the RoPE tails, then the
                ones-matmul and the per-block normalize pieces."""
                fins = list(fins)
                return fins[:1] + list(tails) + fins[1:]

            # ---- software-pipelined head loop ----
            d_fins = []           # finalize of the previous attention group
            d_tails = prev_tails  # RoPE tails of the previous proj chunk
            for h in range(NH):
                if h + 1 < NH:
                    qron = work.tile([P, T], BF16, tag="qro")
                    kron = work.tile([P, T], BF16, tag="kro")
                    ro_next = (qron, kron)
                if h == NH - 2:
                    nc.sync.dma_start(
                        out=wo_sb, in_=wo.rearrange("(h p) t -> p h t", p=P)
                    )
                for g in range(NT // QG):
                    if g == 1 and h + 2 < NH:
                        # prefetch head h+2's weights ~3 groups early so
                        # the first filler matmuls never wait on the DMA
                        w_list[h + 2] = load_w(h + 2)
                    fillers = weave(d_fins, d_tails)
                    d_fins, d_tails = [], []
                    if h + 1 < NH:
                        a_ops, tails = make_stageA(g, w_list[h + 1], ro_next)
                        fillers += a_ops
                        d_tails = tails
                    elif g > 0:
                        fillers += make_wo_ops(g - 1)
                    d_fins = attention_group(h, g, ro_cur, fillers)
                if h + 1 < NH:
                    ro_cur = ro_next
            for op in weave(d_fins, d_tails):
                op()
            for op in make_wo_ops(NT // QG - 1):
                op()

    if split:
        split_sync_waits(nc)
    return nc


BF = ml_dtypes.bfloat16


def host_inputs(x, cos, sin, Wq, Wk, Wv, Wo):
    """Build the 8 per-core input maps from full inputs."""
    cosT = np.ascontiguousarray(cos.T).astype(np.float32)
    sinT = np.ascontiguousarray(sin.T).astype(np.float32)
    rotm = np.zeros((P, P), np.float32)
    half = P // 2
    for m in range(half):
        rotm[m + half, m] = -1.0
        rotm[m, m + half] = 1.0
    ones = np.ones((P, P), np.float32)
    ident = np.eye(P, dtype=np.float32)
    # in [k,q] layout the non-causal region is k>q: strictly lower triangle
    trilT = np.tril(np.full((P, P), -1.0e30, np.float32), k=-1)

    common = {
        "cosk": cosT.astype(BF),
        "sink": sinT.astype(BF),
        "rotm": rotm.astype(BF),
        "ones": ones.astype(BF),
        "ident": ident.astype(BF),
        "trilT": trilT.astype(BF),
    }
    def arrange_w(Wslice):
        # [C, NH*P] (transposed weight) -> [P, NH, NCT, P] so each head's
        # block is contiguous per partition
        wT = np.ascontiguousarray(Wslice.T)  # [C, 512]
        return np.ascontiguousarray(
            wT.reshape(NCT, P, NH, P).transpose(1, 2, 0, 3)
        )

    in_maps = []
    for core in range(8):
        b, g = core // 4, core % 4
        r0 = g * NH * P
        m = dict(common)
        m["xT"] = np.ascontiguousarray(x[b].T).astype(BF)
        m["wq"] = arrange_w(Wq[r0:r0 + NH * P, :]).astype(BF)
        m["wk"] = arrange_w(Wk[r0:r0 + NH * P, :]).astype(BF)
        m["wv"] = np.ascontiguousarray(Wv[r0:r0 + NH * P, :].T).astype(BF)
        m["wo"] = np.ascontiguousarray(Wo[:, r0:r0 + NH * P].T).astype(BF)
        in_maps.append(m)
    return in_maps


def combine_outputs(results):
    ys = [np.asarray(r["y"], np.float32) for r in results]
    return np.stack([ys[0] + ys[1] + ys[2] + ys[3], ys[4] + ys[5] + ys[6] + ys[7]])


def partial_reference(x, cos, sin, Wq, Wk, Wv, Wo, core):
    """Numpy fp32 reference of one core's partial output."""
    b, g = core // 4, core % 4
    r0 = g * NH * P
    xb = x[b].astype(np.float32)
    q = xb @ Wq[r0:r0 + NH * P, :].T  # [T, 512]
    k = xb @ Wk[r0:r0 + NH * P, :].T
    v = xb @ Wv[r0:r0 + NH * P, :].T
    q = q.reshape(T, NH, P)
    k = k.reshape(T, NH, P)
    v = v.reshape(T, NH, P)

    def rot(z):
        return np.concatenate([-z[..., P // 2:], z[..., :P // 2]], axis=-1)

    c = cos[:, None, :]
    s = sin[:, None, :]
    q = q * c + rot(q) * s
    k = k * c + rot(k) * s
    out = np.zeros((T, NH, P), np.float32)
    scale = P ** -0.5
    mask = np.tril(np.ones((T, T), bool))
    for h in range(NH):
        S = (q[:, h] @ k[:, h].T) * scale
        S = np.where(mask, S, -np.inf)
        S = S - S.max(axis=-1, keepdims=True)
        Pm = np.exp(S)
        Pm /= Pm.sum(axis=-1, keepdims=True)
        out[:, h] = Pm @ v[:, h]
    out = out.reshape(T, NH * P)
    return out @ Wo[:, r0:r0 + NH * P].astype(np.float32).T


def kernel(x, cos, sin, mask, Wq, Wk, Wv, Wo):
    from concourse.bass_utils import run_bass_kernel_spmd

    x = np.asarray(x)
    cos = np.asarray(cos)
    sin = np.asarray(sin)
    Wq = np.asarray(Wq)
    Wk = np.asarray(Wk)
    Wv = np.asarray(Wv)
    Wo = np.asarray(Wo)

    nc = build_nc()
    in_maps = host_inputs(x, cos, sin, Wq, Wk, Wv, Wo)
    res = run_bass_kernel_spmd(nc, in_maps, core_ids=list(range(8)))
    return combine_outputs(res.results).astype(np.float32)
